# revision 6
# baseline (speedup 1.0000x reference)
"""DelayAttention Trainium2 kernel (v2).

Data-parallel over batch: B=16 split as 2 batches per core across 8 cores.
Per core, per batch, the sequence is processed in 512-token slices
(tokens = (t, n) pairs, 64 nodes per timestep):

  1. x is pre-cast to bf16 on host; DMA-transpose loads xT [d, tok] directly
     (no PE transposes).
  2. Linears Q/K/V/u as bf16 matmuls (weights stationary, K-chunked over d),
     outputs live transposed [dk, tok].
  3. sim[p, tok] via 10 accumulated bf16 matmuls (M=8) against a sliding
     window of the persistent bf16 UT buffer.
  4. pattern softmax, denominator-free: e = exp(sim); N = c_sum^T @ e
     (unnormalized injection); denominator computed TRANSPOSED as
     denT[tok] = e_chunk^T @ ones via 4 tiny matmuls -> one [128,4]
     reciprocal rd (vs a 3.3us [8,512] reciprocal).
  5. attention with TRANSPOSED scores [k, q]: sc1 = K^T Q, sc2 = N^T Q,
     scores = sc1 + rd[k] * sc2 (rd is per-partition!), done via one
     scalar_tensor_tensor per 64-block. exp without max (scores bounded);
     denominator via an extra ones-column appended to V so the AV matmul
     yields [out | rowsum]; final scale by 1/rowsum per partition.

Host-side prep (tiny, O(params) + one x cast): weight transposes to bf16,
m = patterns@Wm.T+bm reshaped to [dk, (s,p)], c_sum = (patterns@Wc.T+bc)
summed over s, bf16 identity.
"""

import os
import sys

import numpy as np

for _p in ("/opt/trn_rl_repo",):
    if _p not in sys.path and os.path.isdir(_p):
        sys.path.insert(0, _p)

import ml_dtypes  # noqa: E402

import concourse.bass as bass  # noqa: E402
import concourse.mybir as mybir  # noqa: E402
import concourse.tile as tile  # noqa: E402
from concourse import bacc  # noqa: E402

F32 = mybir.dt.float32
BF16 = mybir.dt.bfloat16
AX = mybir.AxisListType.X
AF = mybir.ActivationFunctionType
ALU = mybir.AluOpType

N_CORES = 8
N_NODES = 64          # N
D_MODEL = 256         # D
DK = 128
S_WIN = 10            # window size
N_PAT = 8             # patterns
SL = 512              # tokens per slice
INJ0 = S_WIN * N_NODES  # 640: first injected token


def build_program(Bs: int, T: int) -> bass.Bass:
    TOK = T * N_NODES
    nsl = TOK // SL
    assert TOK % SL == 0
    scale = 1.0 / float(np.sqrt(DK))

    nc = bacc.Bacc("TRN2", target_bir_lowering=False, debug=False)

    x_in = nc.dram_tensor("x", [Bs, T, N_NODES, D_MODEL], BF16, kind="ExternalInput")
    wts = {
        k: nc.dram_tensor(f"wt{k}", [2, 128, DK], BF16, kind="ExternalInput")
        for k in ("q", "k", "v", "u")
    }
    biases_in = {
        k: nc.dram_tensor(f"b{k}", [DK, 1], F32, kind="ExternalInput")
        for k in ("q", "k", "v", "u")
    }
    mT_in = nc.dram_tensor("mT", [DK, S_WIN * N_PAT], BF16, kind="ExternalInput")
    csum_in = nc.dram_tensor("csum", [N_PAT, DK], BF16, kind="ExternalInput")
    idb_in = nc.dram_tensor("idb", [128, 128], BF16, kind="ExternalInput")
    ones81_in = nc.dram_tensor("ones81", [N_PAT, 1], BF16, kind="ExternalInput")
    out_d = nc.dram_tensor("out", [Bs, T, N_NODES, DK], F32, kind="ExternalOutput")

    x_flat = x_in.rearrange("b t n d -> b (t n) d")
    out_flat = out_d.rearrange("b t n d -> b (t n) d")

    with tile.TileContext(nc) as tc:
        with (
            tc.tile_pool(name="consts", bufs=1) as cpool,
            tc.tile_pool(name="stream", bufs=3) as spool,
            tc.tile_pool(name="ut", bufs=1) as utpool,
            tc.tile_pool(name="psL", bufs=2, space="PSUM") as psL,
            tc.tile_pool(name="psP", bufs=2, space="PSUM") as psP,
            tc.tile_pool(name="psN", bufs=1, space="PSUM") as psN,
            tc.tile_pool(name="psV", bufs=1, space="PSUM") as psV,
            tc.tile_pool(name="psT", bufs=2, space="PSUM") as psT,
        ):
            # ---- constants into SBUF ----
            wt_sb = {}
            b_sb = {}
            for k in ("q", "k", "v", "u"):
                wt_sb[k] = cpool.tile([128, 2, DK], BF16, tag=f"wt{k}", name=f"wt{k}_sb")
                nc.sync.dma_start(out=wt_sb[k], in_=wts[k].rearrange("c d m -> d c m"))
                b_sb[k] = cpool.tile([DK, 1], F32, tag=f"b{k}", name=f"b{k}_sb")
                nc.sync.dma_start(out=b_sb[k], in_=biases_in[k][:, :])
            mT_sb = cpool.tile([DK, S_WIN * N_PAT], BF16, tag="mT")
            nc.sync.dma_start(out=mT_sb, in_=mT_in[:, :])
            csum_sb = cpool.tile([N_PAT, DK], BF16, tag="csum")
            nc.sync.dma_start(out=csum_sb, in_=csum_in[:, :])
            idb_sb = cpool.tile([128, 128], BF16, tag="idb")
            nc.sync.dma_start(out=idb_sb, in_=idb_in[:, :])
            ones81_sb = cpool.tile([N_PAT, 1], BF16, tag="ones81")
            nc.sync.dma_start(out=ones81_sb, in_=ones81_in[:, :])

            # Absorb const-DMA semaphores into dedicated PE transposes:
            # walrus's self-loading matmul allows at most 2 sync waits, so
            # real matmuls must never be the first reader of a const DMA.
            def absorb(t):
                p, f = t.shape[0], int(np.prod(t.shape[1:]))
                scr = psT.tile([128, 128], BF16, tag="att", name="absorb_scr")
                nc.tensor.transpose(
                    out=scr[0:f, 0:p], in_=t, identity=idb_sb[0:p, 0:p]
                )

            for k in ("q", "k", "v", "u"):
                for cd in range(2):
                    absorb(wt_sb[k][:, cd, :])
            absorb(mT_sb)
            absorb(csum_sb)
            absorb(ones81_sb)
            absorb(idb_sb)

            for b in range(Bs):
                ut = utpool.tile([128, TOK], BF16, tag="ut")
                for c in range(nsl):
                    tok0 = c * SL
                    # ---- DMA-transposed load: xt chunks [128 d, 512 tok] ----
                    xt = []
                    for cd in range(2):
                        xt_c = spool.tile([128, SL], BF16, tag=f"xt{cd}")
                        nc.sync.dma_start_transpose(
                            out=xt_c,
                            in_=x_flat[
                                b, tok0 : tok0 + SL, cd * 128 : (cd + 1) * 128
                            ],
                        )
                        xt.append(xt_c)

                    def linear(key, tag="lin"):
                        ps = psL.tile([128, SL], F32, tag=tag, name=f"{key}_ps")
                        for cd in range(2):
                            nc.tensor.matmul(
                                ps,
                                lhsT=wt_sb[key][:, cd, :],
                                rhs=xt[cd],
                                start=(cd == 0),
                                stop=(cd == 1),
                            )
                        return ps

                    # ---- u linear -> UT[,:tok] (bf16, +bias) ----
                    u_ps = linear("u")
                    nc.scalar.activation(
                        out=ut[:, tok0 : tok0 + SL],
                        in_=u_ps,
                        func=AF.Identity,
                        bias=b_sb["u"],
                    )

                    # ---- pattern pipeline ----
                    rd4 = None
                    nt_bf = None
                    if c >= 1:
                        j0 = 128 if c == 1 else 0
                        nsim = SL - j0
                        sim_ps = psP.tile([N_PAT, SL], F32, tag="pat", name="sim_ps")
                        for s in range(S_WIN):
                            ucol = tok0 + j0 - INJ0 + 64 * s
                            nc.tensor.matmul(
                                sim_ps[:, j0:],
                                lhsT=mT_sb[:, s * N_PAT : (s + 1) * N_PAT],
                                rhs=ut[:, ucol : ucol + nsim],
                                start=(s == 0),
                                stop=(s == S_WIN - 1),
                            )
                        e_t = spool.tile([N_PAT, SL], BF16, tag="e")
                        if j0 > 0:
                            nc.vector.memset(e_t[:, 0:j0], 0.0)
                        nc.scalar.activation(
                            out=e_t[:, j0:], in_=sim_ps[:, j0:], func=AF.Exp
                        )
                        # unnormalized injection N = csum^T @ e  [128 d, 512]
                        n_ps = psN.tile([128, SL], F32, tag="nt", name="n_ps")
                        nc.tensor.matmul(
                            n_ps, lhsT=csum_sb, rhs=e_t, start=True, stop=True
                        )
                        nt_bf = spool.tile([128, SL], BF16, tag="ntbf")
                        nc.vector.tensor_copy(out=nt_bf, in_=n_ps)
                        # transposed denominator denT[tok] per 128-chunk
                        den4_ps = psP.tile([128, 4], F32, tag="pat", name="den4_ps")
                        for ch in range(4):
                            nc.tensor.matmul(
                                den4_ps[:, ch : ch + 1],
                                lhsT=e_t[:, ch * 128 : (ch + 1) * 128],
                                rhs=ones81_sb,
                                start=True,
                                stop=True,
                            )
                        rd4 = spool.tile([128, 4], F32, tag="rd4")
                        ch0 = j0 // 128
                        if ch0 > 0:
                            nc.vector.memset(rd4[:, 0:ch0], 0.0)
                        nc.vector.reciprocal(
                            out=rd4[:, ch0:], in_=den4_ps[:, ch0:]
                        )

                    # ---- K/Q linears -> bf16 (bias via DVE) ----
                    k_ps = linear("k")
                    kt_bf = spool.tile([128, SL], BF16, tag="kt")
                    nc.vector.tensor_scalar_add(out=kt_bf, in0=k_ps, scalar1=b_sb["k"])
                    q_ps = linear("q")
                    qt_bf = spool.tile([128, SL], BF16, tag="qt")
                    nc.vector.tensor_scalar_add(out=qt_bf, in0=q_ps, scalar1=b_sb["q"])

                    # ---- V linear -> bf16 VT -> transpose to V natural ----
                    v_ps = linear("v")
                    vt_bf = spool.tile([128, SL], BF16, tag="vt")
                    nc.scalar.activation(
                        out=vt_bf, in_=v_ps, func=AF.Identity, bias=b_sb["v"]
                    )
                    # vext[tok, (pr, d|1)]: pair pr rows 0:64 = even t, 64:128 odd
                    vx_ps = psV.tile([128, 4, 132], BF16, tag="vext", name="vx_ps")
                    for pr in range(4):
                        nc.tensor.transpose(
                            out=vx_ps[:, pr, 0:128],
                            in_=vt_bf[:, pr * 128 : (pr + 1) * 128],
                            identity=idb_sb,
                        )
                    vext = spool.tile([128, 4, 132], BF16, tag="vnat")
                    nc.scalar.copy(out=vext[:, :, 0:128], in_=vx_ps[:, :, 0:128])
                    nc.vector.memset(vext[:, :, 128:129], 1.0)

                    # ---- attention: 4 pairs of timesteps, scores [k, q] ----
                    out_sb = spool.tile([128, 4, DK], F32, tag="osb")
                    for pr in range(4):
                        c1 = pr * 128
                        sc1 = psT.tile([128, 128], F32, tag="att", name="sc1")
                        nc.tensor.matmul(
                            sc1,
                            lhsT=kt_bf[:, c1 : c1 + 128],
                            rhs=qt_bf[:, c1 : c1 + 128],
                            start=True,
                            stop=True,
                        )
                        attn_bf = spool.tile([128, 64], BF16, tag="attn")
                        if rd4 is not None:
                            sc2 = psT.tile([128, 128], F32, tag="att", name="sc2")
                            nc.tensor.matmul(
                                sc2,
                                lhsT=nt_bf[:, c1 : c1 + 128],
                                rhs=qt_bf[:, c1 : c1 + 128],
                                start=True,
                                stop=True,
                            )
                            sc2s = spool.tile([128, 64], F32, tag="sc2s")
                            sccmb = spool.tile([128, 64], F32, tag="sccmb")
                            for h in range(2):
                                r0 = 64 * h
                                nc.vector.tensor_scalar_mul(
                                    out=sc2s[r0 : r0 + 64, :],
                                    in0=sc2[r0 : r0 + 64, r0 : r0 + 64],
                                    scalar1=rd4[r0 : r0 + 64, pr : pr + 1],
                                )
                                nc.vector.tensor_tensor(
                                    out=sccmb[r0 : r0 + 64, :],
                                    in0=sc1[r0 : r0 + 64, r0 : r0 + 64],
                                    in1=sc2s[r0 : r0 + 64, :],
                                    op=ALU.add,
                                )
                            nc.scalar.activation(
                                out=attn_bf,
                                in_=sccmb,
                                func=AF.Exp,
                                scale=scale,
                            )
                        else:
                            for h in range(2):
                                r0 = 64 * h
                                nc.scalar.activation(
                                    out=attn_bf[r0 : r0 + 64, :],
                                    in_=sc1[r0 : r0 + 64, r0 : r0 + 64],
                                    func=AF.Exp,
                                    scale=scale,
                                )
                        o_ps = psT.tile([128, 132], F32, tag="att", name="o_ps")
                        nc.tensor.matmul(
                            o_ps[0:64, 0:129],
                            lhsT=attn_bf[0:64, :],
                            rhs=vext[0:64, pr, 0:129],
                            start=True,
                            stop=True,
                        )
                        nc.tensor.matmul(
                            o_ps[64:128, 0:129],
                            lhsT=attn_bf[64:128, :],
                            rhs=vext[64:128, pr, 0:129],
                            start=True,
                            stop=True,
                            tile_position=(64, 64),
                        )
                        rs = spool.tile([128, 1], F32, tag="rs")
                        nc.vector.reciprocal(out=rs, in_=o_ps[:, 128:129])
                        nc.vector.tensor_scalar_mul(
                            out=out_sb[:, pr, :], in0=o_ps[:, 0:128], scalar1=rs
                        )

                    nc.sync.dma_start(
                        out=out_flat[b, tok0 : tok0 + SL, :].rearrange(
                            "(j p) d -> p j d", p=128
                        ),
                        in_=out_sb,
                    )
    nc.finalize()
    return nc


def _host_prep(inputs: dict) -> dict:
    f = np.float32
    bf = ml_dtypes.bfloat16
    aux = {}
    for k, (W, bias) in {
        "q": (inputs["WQ"], inputs["bQ"]),
        "k": (inputs["WK"], inputs["bK"]),
        "v": (inputs["WV"], inputs["bV"]),
        "u": (inputs["Wu"], inputs["bu"]),
    }.items():
        aux[f"wt{k}"] = np.ascontiguousarray(
            np.asarray(W, f).T.reshape(2, 128, DK)
        ).astype(bf)
        aux[f"b{k}"] = np.ascontiguousarray(np.asarray(bias, f).reshape(DK, 1))
    patterns = np.asarray(inputs["patterns"], f)
    m = patterns @ np.asarray(inputs["Wm"], f).T + np.asarray(inputs["bm"], f)
    aux["mT"] = np.ascontiguousarray(
        m.transpose(2, 1, 0).reshape(DK, S_WIN * N_PAT)
    ).astype(bf)
    aux["csum"] = np.ascontiguousarray(
        (patterns @ np.asarray(inputs["Wc"], f).T + np.asarray(inputs["bc"], f)).sum(
            axis=1
        )
    ).astype(bf)
    aux["idb"] = np.eye(128, dtype=bf)
    aux["ones81"] = np.ones([N_PAT, 1], bf)
    return aux


TRACE = False
LAST_RESULTS = None


def kernel(**inputs) -> np.ndarray:
    global LAST_RESULTS
    from concourse.bass_utils import run_bass_kernel_spmd

    x = np.asarray(inputs["x"], np.float32)
    B, T = x.shape[0], x.shape[1]
    bs = B // N_CORES
    x_bf = x.astype(ml_dtypes.bfloat16)
    aux = _host_prep(inputs)
    nc = build_program(bs, T)
    in_maps = [dict(aux, x=x_bf[i * bs : (i + 1) * bs]) for i in range(N_CORES)]
    res = run_bass_kernel_spmd(nc, in_maps, list(range(N_CORES)), trace=TRACE)
    LAST_RESULTS = res
    return np.concatenate([r["out"] for r in res.results], axis=0)


# revision 8
# speedup vs baseline: 1.8599x; 1.8599x over previous
"""DelayAttention Trainium2 kernel (v3).

Data-parallel over batch: B=16 split as 2 batches per core across 8 cores.
Per core, per batch, the sequence is processed in 512-token slices
(tokens = (t, n) pairs, 64 nodes per timestep):

  1. x is pre-cast to bf16 on host; DMA-transpose loads xT [d, tok] directly
     (no PE transposes for x).
  2. Linears Q/K/V/u as bf16 matmuls (weights stationary, K-chunked over d),
     outputs live transposed [dk, tok].
  3. sim[p, tok] via 10 accumulated bf16 matmuls (M=8) against a sliding
     window of the persistent bf16 UT buffer.
  4. pattern softmax, denominator-free: e = exp(sim); N = c_sum^T @ e
     (unnormalized injection); denominator computed TRANSPOSED as
     denT[tok] = e_chunk^T @ ones via 4 tiny matmuls -> one [128,4]
     reciprocal rd.
  5. attention with TRANSPOSED scores [k, q]: sc1 = K^T Q, sc2 = N^T Q,
     scores = sc1 + rd[k] * sc2 (rd is per-partition). exp without max
     (scores bounded); denominator via an extra ones-column appended to V
     so the AV matmul yields [out | rowsum]; final scale by 1/rowsum.
  PSUM (8 banks): lin x2, pat (sim/nt/den4) x2, att (sc1|sc2|o per pair,
  column regions of one bank) x2, vx (V-transpose) x2.
"""

import os
import sys

import numpy as np

for _p in ("/opt/trn_rl_repo",):
    if _p not in sys.path and os.path.isdir(_p):
        sys.path.insert(0, _p)

import ml_dtypes  # noqa: E402

import concourse.bass as bass  # noqa: E402
import concourse.mybir as mybir  # noqa: E402
import concourse.tile as tile  # noqa: E402
from concourse import bacc  # noqa: E402

F32 = mybir.dt.float32
BF16 = mybir.dt.bfloat16
AX = mybir.AxisListType.X
AF = mybir.ActivationFunctionType
ALU = mybir.AluOpType

N_CORES = 8
N_NODES = 64          # N
D_MODEL = 256         # D
DK = 128
S_WIN = 10            # window size
N_PAT = 8             # patterns
SL = 512              # tokens per slice
INJ0 = S_WIN * N_NODES  # 640: first injected token


def build_program(Bs: int, T: int) -> bass.Bass:
    TOK = T * N_NODES
    nsl = TOK // SL
    assert TOK % SL == 0
    scale = 1.0 / float(np.sqrt(DK))

    nc = bacc.Bacc("TRN2", target_bir_lowering=False, debug=False)

    x_in = nc.dram_tensor("x", [Bs, T, N_NODES, D_MODEL], BF16, kind="ExternalInput")
    wts = {
        k: nc.dram_tensor(f"wt{k}", [2, 128, DK], BF16, kind="ExternalInput")
        for k in ("q", "k", "v", "u")
    }
    biases_in = {
        k: nc.dram_tensor(f"b{k}", [DK, 1], F32, kind="ExternalInput")
        for k in ("q", "k", "v", "u")
    }
    mT_in = nc.dram_tensor("mT", [DK, S_WIN * N_PAT], BF16, kind="ExternalInput")
    csum_in = nc.dram_tensor("csum", [N_PAT, DK], BF16, kind="ExternalInput")
    idb_in = nc.dram_tensor("idb", [128, 128], BF16, kind="ExternalInput")
    ones81_in = nc.dram_tensor("ones81", [N_PAT, 1], BF16, kind="ExternalInput")
    out_d = nc.dram_tensor("out", [Bs, T, N_NODES, DK], F32, kind="ExternalOutput")

    x_flat = x_in.rearrange("b t n d -> b (t n) d")
    out_flat = out_d.rearrange("b t n d -> b (t n) d")

    with tile.TileContext(nc) as tc:
        with (
            tc.tile_pool(name="consts", bufs=1) as cpool,
            tc.tile_pool(name="stream", bufs=3) as spool,
            tc.tile_pool(name="ut", bufs=1) as utpool,
            tc.tile_pool(name="psL", bufs=2, space="PSUM") as psL,
            tc.tile_pool(name="psP", bufs=2, space="PSUM") as psP,
            tc.tile_pool(name="psA", bufs=2, space="PSUM") as psA,
            tc.tile_pool(name="psV", bufs=2, space="PSUM") as psV,
        ):
            # ---- constants into SBUF ----
            wt_sb = {}
            b_sb = {}
            for k in ("q", "k", "v", "u"):
                wt_sb[k] = cpool.tile([128, 2, DK], BF16, tag=f"wt{k}", name=f"wt{k}_sb")
                nc.sync.dma_start(out=wt_sb[k], in_=wts[k].rearrange("c d m -> d c m"))
                b_sb[k] = cpool.tile([DK, 1], F32, tag=f"b{k}", name=f"b{k}_sb")
                nc.sync.dma_start(out=b_sb[k], in_=biases_in[k][:, :])
            mT_sb = cpool.tile([DK, S_WIN * N_PAT], BF16, tag="mT")
            nc.sync.dma_start(out=mT_sb, in_=mT_in[:, :])
            csum_sb = cpool.tile([N_PAT, DK], BF16, tag="csum")
            nc.sync.dma_start(out=csum_sb, in_=csum_in[:, :])
            idb_sb = cpool.tile([128, 128], BF16, tag="idb")
            nc.sync.dma_start(out=idb_sb, in_=idb_in[:, :])
            ones81_sb = cpool.tile([N_PAT, 1], BF16, tag="ones81")
            nc.sync.dma_start(out=ones81_sb, in_=ones81_in[:, :])

            # Absorb const-DMA semaphores into dedicated PE transposes:
            # walrus's self-loading matmul allows at most 2 sync waits, so
            # real matmuls must never be the first reader of a const DMA.
            def absorb(t):
                p, f = t.shape[0], int(np.prod(t.shape[1:]))
                scr = psV.tile([128, 4, 132], BF16, tag="vx", name="absorb_scr")
                nc.tensor.transpose(
                    out=scr[0:f, 0, 0:p],
                    in_=t,
                    identity=idb_sb[0:p, 0:p],
                )

            for k in ("q", "k", "v", "u"):
                for cd in range(2):
                    absorb(wt_sb[k][:, cd, :])
            absorb(mT_sb)
            absorb(csum_sb)
            absorb(ones81_sb)
            absorb(idb_sb)

            for b in range(Bs):
                ut = utpool.tile([128, TOK], BF16, tag="ut")
                for c in range(nsl):
                    tok0 = c * SL
                    # ---- DMA-transposed load: xt chunks [128 d, 512 tok] ----
                    xt = []
                    for cd in range(2):
                        xt_c = spool.tile([128, SL], BF16, tag=f"xt{cd}")
                        nc.sync.dma_start_transpose(
                            out=xt_c,
                            in_=x_flat[
                                b, tok0 : tok0 + SL, cd * 128 : (cd + 1) * 128
                            ],
                        )
                        xt.append(xt_c)

                    def linear(key):
                        ps = psL.tile([128, SL], F32, tag="lin", name=f"{key}_ps")
                        for cd in range(2):
                            nc.tensor.matmul(
                                ps,
                                lhsT=wt_sb[key][:, cd, :],
                                rhs=xt[cd],
                                start=(cd == 0),
                                stop=(cd == 1),
                            )
                        return ps

                    # ---- u linear -> UT[,:tok] (bf16, +bias) ----
                    u_ps = linear("u")
                    nc.scalar.activation(
                        out=ut[:, tok0 : tok0 + SL],
                        in_=u_ps,
                        func=AF.Identity,
                        bias=b_sb["u"],
                    )

                    # ---- pattern pipeline ----
                    rd4 = None
                    nt_bf = None
                    if c >= 1:
                        j0 = 128 if c == 1 else 0
                        nsim = SL - j0
                        sim_ps = psP.tile([N_PAT, SL], F32, tag="pat", name="sim_ps")
                        for s in range(S_WIN):
                            ucol = tok0 + j0 - INJ0 + 64 * s
                            nc.tensor.matmul(
                                sim_ps[:, j0:],
                                lhsT=mT_sb[:, s * N_PAT : (s + 1) * N_PAT],
                                rhs=ut[:, ucol : ucol + nsim],
                                start=(s == 0),
                                stop=(s == S_WIN - 1),
                            )
                        e_t = spool.tile([N_PAT, SL], BF16, tag="e")
                        if j0 > 0:
                            nc.vector.memset(e_t[:, 0:j0], 0.0)
                        nc.scalar.activation(
                            out=e_t[:, j0:], in_=sim_ps[:, j0:], func=AF.Exp
                        )
                        # unnormalized injection N = csum^T @ e  [128 d, 512]
                        n_ps = psP.tile([128, SL], F32, tag="pat", name="n_ps")
                        nc.tensor.matmul(
                            n_ps, lhsT=csum_sb, rhs=e_t, start=True, stop=True
                        )
                        nt_bf = spool.tile([128, SL], BF16, tag="ntbf")
                        nc.vector.tensor_copy(out=nt_bf, in_=n_ps)
                        # transposed denominator denT[tok] per 128-chunk
                        den4_ps = psP.tile([128, 4], F32, tag="pat", name="den4_ps")
                        for ch in range(4):
                            nc.tensor.matmul(
                                den4_ps[:, ch : ch + 1],
                                lhsT=e_t[:, ch * 128 : (ch + 1) * 128],
                                rhs=ones81_sb,
                                start=True,
                                stop=True,
                            )
                        rd4 = spool.tile([128, 4], F32, tag="rd4")
                        ch0 = j0 // 128
                        if ch0 > 0:
                            nc.vector.memset(rd4[:, 0:ch0], 0.0)
                        nc.vector.reciprocal(
                            out=rd4[:, ch0:], in_=den4_ps[:, ch0:]
                        )

                    # ---- K (DVE bias) / Q (ACT bias) linears -> bf16 ----
                    k_ps = linear("k")
                    kt_bf = spool.tile([128, SL], BF16, tag="kt")
                    nc.vector.tensor_scalar_add(out=kt_bf, in0=k_ps, scalar1=b_sb["k"])
                    q_ps = linear("q")
                    qt_bf = spool.tile([128, SL], BF16, tag="qt")
                    nc.scalar.activation(
                        out=qt_bf, in_=q_ps, func=AF.Identity, bias=b_sb["q"]
                    )

                    # ---- V linear -> bf16 VT -> transpose to V natural ----
                    v_ps = linear("v")
                    vt_bf = spool.tile([128, SL], BF16, tag="vt")
                    nc.scalar.activation(
                        out=vt_bf, in_=v_ps, func=AF.Identity, bias=b_sb["v"]
                    )
                    # vext[tok, (pr, d|1)]: pair pr rows 0:64 = even t, 64:128 odd
                    vx_ps = psV.tile([128, 4, 132], BF16, tag="vx", name="vx_ps")
                    for pr in range(4):
                        nc.tensor.transpose(
                            out=vx_ps[:, pr, 0:128],
                            in_=vt_bf[:, pr * 128 : (pr + 1) * 128],
                            identity=idb_sb,
                        )
                    vext = spool.tile([128, 4, 132], BF16, tag="vnat")
                    nc.scalar.copy(out=vext[:, :, 0:128], in_=vx_ps[:, :, 0:128])
                    nc.vector.memset(vext[:, :, 128:129], 1.0)

                    # ---- attention: 4 pairs of timesteps, scores [k, q] ----
                    # att tile regions: [0:128]=sc1, [128:256]=sc2, [256:385]=o|den
                    out_sb = spool.tile([128, 4, DK], F32, tag="osb")
                    for pr in range(4):
                        c1 = pr * 128
                        att = psA.tile([128, 512], F32, tag="att", name="att")
                        nc.tensor.matmul(
                            att[:, 0:128],
                            lhsT=kt_bf[:, c1 : c1 + 128],
                            rhs=qt_bf[:, c1 : c1 + 128],
                            start=True,
                            stop=True,
                        )
                        attn_bf = spool.tile([128, 64], BF16, tag="attn")
                        if rd4 is not None:
                            nc.tensor.matmul(
                                att[:, 128:256],
                                lhsT=nt_bf[:, c1 : c1 + 128],
                                rhs=qt_bf[:, c1 : c1 + 128],
                                start=True,
                                stop=True,
                            )
                            sc2s = spool.tile([128, 128], BF16, tag="sc2s")
                            nc.vector.tensor_scalar_mul(
                                out=sc2s,
                                in0=att[:, 128:256],
                                scalar1=rd4[:, pr : pr + 1],
                            )
                            sccmb = spool.tile([128, 128], BF16, tag="sccmb")
                            nc.vector.tensor_tensor(
                                out=sccmb,
                                in0=att[:, 0:128],
                                in1=sc2s,
                                op=ALU.add,
                            )
                            for h in range(2):
                                r0 = 64 * h
                                nc.scalar.activation(
                                    out=attn_bf[r0 : r0 + 64, :],
                                    in_=sccmb[r0 : r0 + 64, r0 : r0 + 64],
                                    func=AF.Exp,
                                    scale=scale,
                                )
                        else:
                            for h in range(2):
                                r0 = 64 * h
                                nc.scalar.activation(
                                    out=attn_bf[r0 : r0 + 64, :],
                                    in_=att[r0 : r0 + 64, r0 : r0 + 64],
                                    func=AF.Exp,
                                    scale=scale,
                                )
                        nc.tensor.matmul(
                            att[0:64, 256:385],
                            lhsT=attn_bf[0:64, :],
                            rhs=vext[0:64, pr, 0:129],
                            start=True,
                            stop=True,
                        )
                        nc.tensor.matmul(
                            att[64:128, 256:385],
                            lhsT=attn_bf[64:128, :],
                            rhs=vext[64:128, pr, 0:129],
                            start=True,
                            stop=True,
                            tile_position=(64, 64),
                        )
                        rs = spool.tile([128, 1], F32, tag="rs")
                        nc.vector.reciprocal(out=rs, in_=att[:, 384:385])
                        nc.vector.tensor_scalar_mul(
                            out=out_sb[:, pr, :], in0=att[:, 256:384], scalar1=rs
                        )

                    nc.sync.dma_start(
                        out=out_flat[b, tok0 : tok0 + SL, :].rearrange(
                            "(j p) d -> p j d", p=128
                        ),
                        in_=out_sb,
                    )
    nc.finalize()
    return nc


def _host_prep(inputs: dict) -> dict:
    f = np.float32
    bf = ml_dtypes.bfloat16
    aux = {}
    for k, (W, bias) in {
        "q": (inputs["WQ"], inputs["bQ"]),
        "k": (inputs["WK"], inputs["bK"]),
        "v": (inputs["WV"], inputs["bV"]),
        "u": (inputs["Wu"], inputs["bu"]),
    }.items():
        aux[f"wt{k}"] = np.ascontiguousarray(
            np.asarray(W, f).T.reshape(2, 128, DK)
        ).astype(bf)
        aux[f"b{k}"] = np.ascontiguousarray(np.asarray(bias, f).reshape(DK, 1))
    patterns = np.asarray(inputs["patterns"], f)
    m = patterns @ np.asarray(inputs["Wm"], f).T + np.asarray(inputs["bm"], f)
    aux["mT"] = np.ascontiguousarray(
        m.transpose(2, 1, 0).reshape(DK, S_WIN * N_PAT)
    ).astype(bf)
    aux["csum"] = np.ascontiguousarray(
        (patterns @ np.asarray(inputs["Wc"], f).T + np.asarray(inputs["bc"], f)).sum(
            axis=1
        )
    ).astype(bf)
    aux["idb"] = np.eye(128, dtype=bf)
    aux["ones81"] = np.ones([N_PAT, 1], bf)
    return aux


TRACE = False
LAST_RESULTS = None


def kernel(**inputs) -> np.ndarray:
    global LAST_RESULTS
    from concourse.bass_utils import run_bass_kernel_spmd

    x = np.asarray(inputs["x"], np.float32)
    B, T = x.shape[0], x.shape[1]
    bs = B // N_CORES
    x_bf = x.astype(ml_dtypes.bfloat16)
    aux = _host_prep(inputs)
    nc = build_program(bs, T)
    in_maps = [dict(aux, x=x_bf[i * bs : (i + 1) * bs]) for i in range(N_CORES)]
    res = run_bass_kernel_spmd(nc, in_maps, list(range(N_CORES)), trace=TRACE)
    LAST_RESULTS = res
    return np.concatenate([r["out"] for r in res.results], axis=0)


# revision 11
# speedup vs baseline: 2.0082x; 1.0797x over previous
"""DelayAttention Trainium2 kernel (v3).

Data-parallel over batch: B=16 split as 2 batches per core across 8 cores.
Per core, per batch, the sequence is processed in 512-token slices
(tokens = (t, n) pairs, 64 nodes per timestep):

  1. x is pre-cast to bf16 on host; DMA-transpose loads xT [d, tok] directly
     (no PE transposes for x).
  2. Linears Q/K/V/u as bf16 matmuls (weights stationary, K-chunked over d),
     outputs live transposed [dk, tok].
  3. sim[p, tok] via 10 accumulated bf16 matmuls (M=8) against a sliding
     window of the persistent bf16 UT buffer.
  4. pattern softmax, denominator-free: e = exp(sim); N = c_sum^T @ e
     (unnormalized injection); denominator computed TRANSPOSED as
     denT[tok] = e_chunk^T @ ones via 4 tiny matmuls -> one [128,4]
     reciprocal rd.
  5. attention with TRANSPOSED scores [k, q]: sc1 = K^T Q, sc2 = N^T Q,
     scores = sc1 + rd[k] * sc2 (rd is per-partition). exp without max
     (scores bounded); denominator via an extra ones-column appended to V
     so the AV matmul yields [out | rowsum]; final scale by 1/rowsum.
  PSUM (8 banks): lin x2, pat (sim/nt/den4) x2, att (sc1|sc2|o per pair,
  column regions of one bank) x2, vx (V-transpose) x2.
"""

import os
import sys

import numpy as np

for _p in ("/opt/trn_rl_repo",):
    if _p not in sys.path and os.path.isdir(_p):
        sys.path.insert(0, _p)

import ml_dtypes  # noqa: E402

import concourse.bass as bass  # noqa: E402
import concourse.mybir as mybir  # noqa: E402
import concourse.tile as tile  # noqa: E402
from concourse import bacc  # noqa: E402

F32 = mybir.dt.float32
BF16 = mybir.dt.bfloat16
AX = mybir.AxisListType.X
AF = mybir.ActivationFunctionType
ALU = mybir.AluOpType

N_CORES = 8
N_NODES = 64          # N
D_MODEL = 256         # D
DK = 128
S_WIN = 10            # window size
N_PAT = 8             # patterns
SL = 512              # tokens per slice
INJ0 = S_WIN * N_NODES  # 640: first injected token


def build_program(Bs: int, T: int) -> bass.Bass:
    TOK = T * N_NODES
    nsl = TOK // SL
    assert TOK % SL == 0
    scale = 1.0 / float(np.sqrt(DK))

    nc = bacc.Bacc("TRN2", target_bir_lowering=False, debug=False)

    x_in = nc.dram_tensor("x", [Bs, T, N_NODES, D_MODEL], BF16, kind="ExternalInput")
    wts = {
        k: nc.dram_tensor(f"wt{k}", [2, 128, DK], BF16, kind="ExternalInput")
        for k in ("q", "k", "v", "u")
    }
    biases_in = {
        k: nc.dram_tensor(f"b{k}", [DK, 1], F32, kind="ExternalInput")
        for k in ("q", "k", "v", "u")
    }
    mT_in = nc.dram_tensor("mT", [DK, S_WIN * N_PAT], BF16, kind="ExternalInput")
    csum_in = nc.dram_tensor("csum", [N_PAT, DK], BF16, kind="ExternalInput")
    idb_in = nc.dram_tensor("idb", [128, 128], BF16, kind="ExternalInput")
    ones81_in = nc.dram_tensor("ones81", [N_PAT, 1], BF16, kind="ExternalInput")
    out_d = nc.dram_tensor("out", [Bs, T, N_NODES, DK], F32, kind="ExternalOutput")

    x_flat = x_in.rearrange("b t n d -> b (t n) d")
    out_flat = out_d.rearrange("b t n d -> b (t n) d")

    with tile.TileContext(nc) as tc:
        with (
            tc.tile_pool(name="consts", bufs=1) as cpool,
            tc.tile_pool(name="stream", bufs=3) as spool,
            tc.tile_pool(name="ut", bufs=1) as utpool,
            tc.tile_pool(name="psL", bufs=3, space="PSUM") as psL,
            tc.tile_pool(name="psP", bufs=2, space="PSUM") as psP,
            tc.tile_pool(name="psA", bufs=2, space="PSUM") as psA,
            tc.tile_pool(name="psV", bufs=1, space="PSUM") as psV,
        ):
            # ---- constants into SBUF ----
            wt_sb = {}
            b_sb = {}
            for k in ("q", "k", "v", "u"):
                wt_sb[k] = cpool.tile([128, 2, DK], BF16, tag=f"wt{k}", name=f"wt{k}_sb")
                nc.sync.dma_start(out=wt_sb[k], in_=wts[k].rearrange("c d m -> d c m"))
                b_sb[k] = cpool.tile([DK, 1], F32, tag=f"b{k}", name=f"b{k}_sb")
                nc.sync.dma_start(out=b_sb[k], in_=biases_in[k][:, :])
            mT_sb = cpool.tile([DK, S_WIN * N_PAT], BF16, tag="mT")
            nc.sync.dma_start(out=mT_sb, in_=mT_in[:, :])
            csum_sb = cpool.tile([N_PAT, DK], BF16, tag="csum")
            nc.sync.dma_start(out=csum_sb, in_=csum_in[:, :])
            idb_sb = cpool.tile([128, 128], BF16, tag="idb")
            nc.sync.dma_start(out=idb_sb, in_=idb_in[:, :])
            ones81_sb = cpool.tile([N_PAT, 1], BF16, tag="ones81")
            nc.sync.dma_start(out=ones81_sb, in_=ones81_in[:, :])

            # Absorb const-DMA semaphores into dedicated PE transposes:
            # walrus's self-loading matmul allows at most 2 sync waits, so
            # real matmuls must never be the first reader of a const DMA.
            def absorb(t):
                p, f = t.shape[0], int(np.prod(t.shape[1:]))
                scr = psV.tile([128, 4, 132], BF16, tag="vx", name="absorb_scr")
                nc.tensor.transpose(
                    out=scr[0:f, 0, 0:p],
                    in_=t,
                    identity=idb_sb[0:p, 0:p],
                )

            for k in ("q", "k", "v", "u"):
                for cd in range(2):
                    absorb(wt_sb[k][:, cd, :])
            absorb(mT_sb)
            absorb(csum_sb)
            absorb(ones81_sb)
            absorb(idb_sb)

            # Pre-zeroed attention-weight ring: exp writes only the diagonal
            # 64x64 blocks, so the off-diagonal blocks stay zero and ONE
            # K=128 block-diagonal AV matmul per pair replaces two K=64 ones.
            attn_ring = []
            for zi in range(3):
                az = cpool.tile([128, 128], BF16, tag=f"az{zi}", name=f"attn_z{zi}")
                nc.vector.memset(az, 0.0)
                attn_ring.append(az)

            for b in range(Bs):
                ut = utpool.tile([128, TOK], BF16, tag="ut")
                for c in range(nsl):
                    tok0 = c * SL
                    # ---- DMA-transposed load: xt chunks [128 d, 512 tok] ----
                    xt = []
                    for cd in range(2):
                        xt_c = spool.tile([128, SL], BF16, tag=f"xt{cd}")
                        nc.sync.dma_start_transpose(
                            out=xt_c,
                            in_=x_flat[
                                b, tok0 : tok0 + SL, cd * 128 : (cd + 1) * 128
                            ],
                        )
                        xt.append(xt_c)

                    def linear(key):
                        ps = psL.tile([128, SL], F32, tag="lin", name=f"{key}_ps")
                        for cd in range(2):
                            nc.tensor.matmul(
                                ps,
                                lhsT=wt_sb[key][:, cd, :],
                                rhs=xt[cd],
                                start=(cd == 0),
                                stop=(cd == 1),
                            )
                        return ps

                    # ---- u linear -> UT[,:tok] (bf16, +bias) ----
                    u_ps = linear("u")
                    nc.scalar.activation(
                        out=ut[:, tok0 : tok0 + SL],
                        in_=u_ps,
                        func=AF.Identity,
                        bias=b_sb["u"],
                    )

                    # ---- pattern pipeline ----
                    rd4 = None
                    nt_bf = None
                    if c >= 1:
                        j0 = 128 if c == 1 else 0
                        nsim = SL - j0
                        sim_ps = psP.tile([N_PAT, SL], F32, tag="pat", name="sim_ps")
                        for s in range(S_WIN):
                            ucol = tok0 + j0 - INJ0 + 64 * s
                            nc.tensor.matmul(
                                sim_ps[:, j0:],
                                lhsT=mT_sb[:, s * N_PAT : (s + 1) * N_PAT],
                                rhs=ut[:, ucol : ucol + nsim],
                                start=(s == 0),
                                stop=(s == S_WIN - 1),
                            )
                        e_t = spool.tile([N_PAT, SL], BF16, tag="e")
                        if j0 > 0:
                            nc.vector.memset(e_t[:, 0:j0], 0.0)
                        nc.scalar.activation(
                            out=e_t[:, j0:], in_=sim_ps[:, j0:], func=AF.Exp
                        )
                        # unnormalized injection N = csum^T @ e  [128 d, 512]
                        n_ps = psP.tile([128, SL], F32, tag="pat", name="n_ps")
                        nc.tensor.matmul(
                            n_ps, lhsT=csum_sb, rhs=e_t, start=True, stop=True
                        )
                        nt_bf = spool.tile([128, SL], BF16, tag="ntbf")
                        nc.vector.tensor_copy(out=nt_bf, in_=n_ps)
                        # transposed denominator denT[tok] per 128-chunk
                        den4_ps = psP.tile([128, 4], F32, tag="pat", name="den4_ps")
                        for ch in range(4):
                            nc.tensor.matmul(
                                den4_ps[:, ch : ch + 1],
                                lhsT=e_t[:, ch * 128 : (ch + 1) * 128],
                                rhs=ones81_sb,
                                start=True,
                                stop=True,
                            )
                        rd4 = spool.tile([128, 4], F32, tag="rd4")
                        ch0 = j0 // 128
                        if ch0 > 0:
                            nc.vector.memset(rd4[:, 0:ch0], 0.0)
                        nc.vector.reciprocal(
                            out=rd4[:, ch0:], in_=den4_ps[:, ch0:]
                        )

                    # ---- K (DVE bias) / Q (ACT bias) linears -> bf16 ----
                    k_ps = linear("k")
                    kt_bf = spool.tile([128, SL], BF16, tag="kt")
                    nc.vector.tensor_scalar_add(out=kt_bf, in0=k_ps, scalar1=b_sb["k"])
                    q_ps = linear("q")
                    qt_bf = spool.tile([128, SL], BF16, tag="qt")
                    nc.scalar.activation(
                        out=qt_bf, in_=q_ps, func=AF.Identity, bias=b_sb["q"]
                    )

                    # ---- V linear -> bf16 VT -> transpose to V natural ----
                    v_ps = linear("v")
                    vt_bf = spool.tile([128, SL], BF16, tag="vt")
                    nc.scalar.activation(
                        out=vt_bf, in_=v_ps, func=AF.Identity, bias=b_sb["v"]
                    )
                    # vext[tok, (pr, d|1)]: pair pr rows 0:64 = even t, 64:128 odd
                    vx_ps = psV.tile([128, 4, 132], BF16, tag="vx", name="vx_ps")
                    for pr in range(4):
                        nc.tensor.transpose(
                            out=vx_ps[:, pr, 0:128],
                            in_=vt_bf[:, pr * 128 : (pr + 1) * 128],
                            identity=idb_sb,
                        )
                    vext = spool.tile([128, 4, 132], BF16, tag="vnat")
                    nc.scalar.copy(out=vext[:, :, 0:128], in_=vx_ps[:, :, 0:128])
                    nc.vector.memset(vext[:, :, 128:129], 1.0)

                    # ---- attention: 4 pairs of timesteps, scores [k, q] ----
                    # att tile regions: [0:128]=sc1, [128:256]=sc2, [256:385]=o|den
                    out_sb = spool.tile([128, 4, DK], F32, tag="osb")
                    for pr in range(4):
                        c1 = pr * 128
                        att = psA.tile([128, 512], F32, tag="att", name="att")
                        nc.tensor.matmul(
                            att[:, 0:128],
                            lhsT=kt_bf[:, c1 : c1 + 128],
                            rhs=qt_bf[:, c1 : c1 + 128],
                            start=True,
                            stop=True,
                        )
                        attn_bf = attn_ring[(c * 4 + pr) % 3]
                        if rd4 is not None:
                            nc.tensor.matmul(
                                att[:, 128:256],
                                lhsT=nt_bf[:, c1 : c1 + 128],
                                rhs=qt_bf[:, c1 : c1 + 128],
                                start=True,
                                stop=True,
                            )
                            sc2s = spool.tile([128, 128], BF16, tag="sc2s")
                            nc.vector.tensor_scalar_mul(
                                out=sc2s,
                                in0=att[:, 128:256],
                                scalar1=rd4[:, pr : pr + 1],
                            )
                            sccmb = spool.tile([128, 128], BF16, tag="sccmb")
                            nc.vector.tensor_tensor(
                                out=sccmb,
                                in0=att[:, 0:128],
                                in1=sc2s,
                                op=ALU.add,
                            )
                            for h in range(2):
                                r0 = 64 * h
                                nc.scalar.activation(
                                    out=attn_bf[r0 : r0 + 64, r0 : r0 + 64],
                                    in_=sccmb[r0 : r0 + 64, r0 : r0 + 64],
                                    func=AF.Exp,
                                    scale=scale,
                                )
                        else:
                            for h in range(2):
                                r0 = 64 * h
                                nc.scalar.activation(
                                    out=attn_bf[r0 : r0 + 64, r0 : r0 + 64],
                                    in_=att[r0 : r0 + 64, r0 : r0 + 64],
                                    func=AF.Exp,
                                    scale=scale,
                                )
                        nc.tensor.matmul(
                            att[:, 256:385],
                            lhsT=attn_bf,
                            rhs=vext[:, pr, 0:129],
                            start=True,
                            stop=True,
                        )
                        rs = spool.tile([128, 1], F32, tag="rs")
                        nc.vector.reciprocal(out=rs, in_=att[:, 384:385])
                        nc.vector.tensor_scalar_mul(
                            out=out_sb[:, pr, :], in0=att[:, 256:384], scalar1=rs
                        )

                    nc.sync.dma_start(
                        out=out_flat[b, tok0 : tok0 + SL, :].rearrange(
                            "(j p) d -> p j d", p=128
                        ),
                        in_=out_sb,
                    )
    nc.finalize()
    return nc


def _host_prep(inputs: dict) -> dict:
    f = np.float32
    bf = ml_dtypes.bfloat16
    aux = {}
    for k, (W, bias) in {
        "q": (inputs["WQ"], inputs["bQ"]),
        "k": (inputs["WK"], inputs["bK"]),
        "v": (inputs["WV"], inputs["bV"]),
        "u": (inputs["Wu"], inputs["bu"]),
    }.items():
        aux[f"wt{k}"] = np.ascontiguousarray(
            np.asarray(W, f).T.reshape(2, 128, DK)
        ).astype(bf)
        aux[f"b{k}"] = np.ascontiguousarray(np.asarray(bias, f).reshape(DK, 1))
    patterns = np.asarray(inputs["patterns"], f)
    m = patterns @ np.asarray(inputs["Wm"], f).T + np.asarray(inputs["bm"], f)
    aux["mT"] = np.ascontiguousarray(
        m.transpose(2, 1, 0).reshape(DK, S_WIN * N_PAT)
    ).astype(bf)
    aux["csum"] = np.ascontiguousarray(
        (patterns @ np.asarray(inputs["Wc"], f).T + np.asarray(inputs["bc"], f)).sum(
            axis=1
        )
    ).astype(bf)
    aux["idb"] = np.eye(128, dtype=bf)
    aux["ones81"] = np.ones([N_PAT, 1], bf)
    return aux


TRACE = False
LAST_RESULTS = None


def kernel(**inputs) -> np.ndarray:
    global LAST_RESULTS
    from concourse.bass_utils import run_bass_kernel_spmd

    x = np.asarray(inputs["x"], np.float32)
    B, T = x.shape[0], x.shape[1]
    bs = B // N_CORES
    x_bf = x.astype(ml_dtypes.bfloat16)
    aux = _host_prep(inputs)
    nc = build_program(bs, T)
    in_maps = [dict(aux, x=x_bf[i * bs : (i + 1) * bs]) for i in range(N_CORES)]
    res = run_bass_kernel_spmd(nc, in_maps, list(range(N_CORES)), trace=TRACE)
    LAST_RESULTS = res
    return np.concatenate([r["out"] for r in res.results], axis=0)


# revision 14
# speedup vs baseline: 2.0216x; 1.0067x over previous
"""DelayAttention Trainium2 kernel (v3).

Data-parallel over batch: B=16 split as 2 batches per core across 8 cores.
Per core, per batch, the sequence is processed in 512-token slices
(tokens = (t, n) pairs, 64 nodes per timestep):

  1. x is pre-cast to bf16 on host; DMA-transpose loads xT [d, tok] directly
     (no PE transposes for x).
  2. Linears Q/K/V/u as bf16 matmuls (weights stationary, K-chunked over d),
     outputs live transposed [dk, tok].
  3. sim[p, tok] via 10 accumulated bf16 matmuls (M=8) against a sliding
     window of the persistent bf16 UT buffer.
  4. pattern softmax, denominator-free: e = exp(sim); N = c_sum^T @ e
     (unnormalized injection); denominator computed TRANSPOSED as
     denT[tok] = e_chunk^T @ ones via 4 tiny matmuls -> one [128,4]
     reciprocal rd.
  5. attention with TRANSPOSED scores [k, q]: sc1 = K^T Q, sc2 = N^T Q,
     scores = sc1 + rd[k] * sc2 (rd is per-partition). exp without max
     (scores bounded); denominator via an extra ones-column appended to V
     so the AV matmul yields [out | rowsum]; final scale by 1/rowsum.
  PSUM (8 banks): lin x2, pat (sim/nt/den4) x2, att (sc1|sc2|o per pair,
  column regions of one bank) x2, vx (V-transpose) x2.
"""

import os
import sys

import numpy as np

for _p in ("/opt/trn_rl_repo",):
    if _p not in sys.path and os.path.isdir(_p):
        sys.path.insert(0, _p)

import ml_dtypes  # noqa: E402

import concourse.bass as bass  # noqa: E402
import concourse.mybir as mybir  # noqa: E402
import concourse.tile as tile  # noqa: E402
from concourse import bacc  # noqa: E402

F32 = mybir.dt.float32
BF16 = mybir.dt.bfloat16
AX = mybir.AxisListType.X
AF = mybir.ActivationFunctionType
ALU = mybir.AluOpType

N_CORES = 8
N_NODES = 64          # N
D_MODEL = 256         # D
DK = 128
S_WIN = 10            # window size
N_PAT = 8             # patterns
SL = 512              # tokens per slice
INJ0 = S_WIN * N_NODES  # 640: first injected token


def build_program(Bs: int, T: int) -> bass.Bass:
    TOK = T * N_NODES
    nsl = TOK // SL
    assert TOK % SL == 0
    scale = 1.0 / float(np.sqrt(DK))

    nc = bacc.Bacc("TRN2", target_bir_lowering=False, debug=False)

    x_in = nc.dram_tensor("x", [Bs, T, N_NODES, D_MODEL], BF16, kind="ExternalInput")
    wts = {
        k: nc.dram_tensor(f"wt{k}", [2, 128, DK], BF16, kind="ExternalInput")
        for k in ("q", "k", "v", "u")
    }
    biases_in = {
        k: nc.dram_tensor(f"b{k}", [DK, 1], F32, kind="ExternalInput")
        for k in ("q", "k", "v", "u")
    }
    mT_in = nc.dram_tensor("mT", [DK, S_WIN * N_PAT], BF16, kind="ExternalInput")
    csum_in = nc.dram_tensor("csum", [N_PAT, DK], BF16, kind="ExternalInput")
    idb_in = nc.dram_tensor("idb", [128, 128], BF16, kind="ExternalInput")
    ones81_in = nc.dram_tensor("ones81", [N_PAT, 1], BF16, kind="ExternalInput")
    out_d = nc.dram_tensor("out", [Bs, T, N_NODES, DK], F32, kind="ExternalOutput")

    x_flat = x_in.rearrange("b t n d -> b (t n) d")
    out_flat = out_d.rearrange("b t n d -> b (t n) d")

    with tile.TileContext(nc) as tc:
        with (
            tc.tile_pool(name="consts", bufs=1) as cpool,
            tc.tile_pool(name="stream", bufs=3) as spool,
            tc.tile_pool(name="ut", bufs=1) as utpool,
            tc.tile_pool(name="psL", bufs=3, space="PSUM") as psL,
            tc.tile_pool(name="psP", bufs=2, space="PSUM") as psP,
            tc.tile_pool(name="psA", bufs=2, space="PSUM") as psA,
            tc.tile_pool(name="psV", bufs=1, space="PSUM") as psV,
        ):
            # ---- constants into SBUF ----
            wt_sb = {}
            b_sb = {}
            for k in ("q", "k", "v", "u"):
                wt_sb[k] = cpool.tile([128, 2, DK], BF16, tag=f"wt{k}", name=f"wt{k}_sb")
                nc.sync.dma_start(out=wt_sb[k], in_=wts[k].rearrange("c d m -> d c m"))
                b_sb[k] = cpool.tile([DK, 1], F32, tag=f"b{k}", name=f"b{k}_sb")
                nc.sync.dma_start(out=b_sb[k], in_=biases_in[k][:, :])
            mT_sb = cpool.tile([DK, S_WIN * N_PAT], BF16, tag="mT")
            nc.sync.dma_start(out=mT_sb, in_=mT_in[:, :])
            csum_sb = cpool.tile([N_PAT, DK], BF16, tag="csum")
            nc.sync.dma_start(out=csum_sb, in_=csum_in[:, :])
            idb_sb = cpool.tile([128, 128], BF16, tag="idb")
            nc.sync.dma_start(out=idb_sb, in_=idb_in[:, :])
            ones81_sb = cpool.tile([N_PAT, 1], BF16, tag="ones81")
            nc.sync.dma_start(out=ones81_sb, in_=ones81_in[:, :])

            # Absorb const-DMA semaphores into dedicated PE transposes:
            # walrus's self-loading matmul allows at most 2 sync waits, so
            # real matmuls must never be the first reader of a const DMA.
            def absorb(t):
                p, f = t.shape[0], int(np.prod(t.shape[1:]))
                scr = psV.tile([128, 4, 132], BF16, tag="vx", name="absorb_scr")
                nc.tensor.transpose(
                    out=scr[0:f, 0, 0:p],
                    in_=t,
                    identity=idb_sb[0:p, 0:p],
                )

            for k in ("q", "k", "v", "u"):
                for cd in range(2):
                    absorb(wt_sb[k][:, cd, :])
            absorb(mT_sb)
            absorb(csum_sb)
            absorb(ones81_sb)
            absorb(idb_sb)

            # Pre-zeroed attention-weight ring: exp writes only the diagonal
            # 64x64 blocks, so the off-diagonal blocks stay zero and ONE
            # K=128 block-diagonal AV matmul per pair replaces two K=64 ones.
            attn_ring = []
            for zi in range(3):
                az = cpool.tile([128, 128], BF16, tag=f"az{zi}", name=f"attn_z{zi}")
                nc.vector.memset(az, 0.0)
                attn_ring.append(az)

            for b in range(Bs):
                ut = utpool.tile([128, TOK], BF16, tag="ut")
                for c in range(nsl):
                    tok0 = c * SL
                    # ---- DMA-transposed load: xt chunks [128 d, 512 tok] ----
                    xt = []
                    for cd in range(2):
                        xt_c = spool.tile([128, SL], BF16, tag=f"xt{cd}")
                        nc.sync.dma_start_transpose(
                            out=xt_c,
                            in_=x_flat[
                                b, tok0 : tok0 + SL, cd * 128 : (cd + 1) * 128
                            ],
                        )
                        xt.append(xt_c)

                    def linear(key):
                        ps = psL.tile([128, SL], F32, tag="lin", name=f"{key}_ps")
                        for cd in range(2):
                            nc.tensor.matmul(
                                ps,
                                lhsT=wt_sb[key][:, cd, :],
                                rhs=xt[cd],
                                start=(cd == 0),
                                stop=(cd == 1),
                            )
                        return ps

                    # ---- u linear -> UT[,:tok] (bf16, +bias) ----
                    u_ps = linear("u")
                    nc.scalar.activation(
                        out=ut[:, tok0 : tok0 + SL],
                        in_=u_ps,
                        func=AF.Identity,
                        bias=b_sb["u"],
                    )

                    # ---- pattern pipeline ----
                    rd4 = None
                    nt_bf = None
                    if c >= 1:
                        j0 = 128 if c == 1 else 0
                        nsim = SL - j0
                        sim_ps = psP.tile([N_PAT, SL], F32, tag="pat", name="sim_ps")
                        for s in range(S_WIN):
                            ucol = tok0 + j0 - INJ0 + 64 * s
                            nc.tensor.matmul(
                                sim_ps[:, j0:],
                                lhsT=mT_sb[:, s * N_PAT : (s + 1) * N_PAT],
                                rhs=ut[:, ucol : ucol + nsim],
                                start=(s == 0),
                                stop=(s == S_WIN - 1),
                            )
                        e_t = spool.tile([N_PAT, SL], BF16, tag="e")
                        if j0 > 0:
                            nc.vector.memset(e_t[:, 0:j0], 0.0)
                        nc.scalar.activation(
                            out=e_t[:, j0:], in_=sim_ps[:, j0:], func=AF.Exp
                        )
                        # unnormalized injection N = csum^T @ e  [128 d, 512]
                        n_ps = psP.tile([128, SL], F32, tag="pat", name="n_ps")
                        nc.tensor.matmul(
                            n_ps, lhsT=csum_sb, rhs=e_t, start=True, stop=True
                        )
                        nt_bf = spool.tile([128, SL], BF16, tag="ntbf")
                        nc.vector.tensor_copy(out=nt_bf, in_=n_ps)
                        # transposed denominator denT[tok] per 128-chunk
                        den4_ps = psP.tile([128, 4], F32, tag="pat", name="den4_ps")
                        for ch in range(4):
                            nc.tensor.matmul(
                                den4_ps[:, ch : ch + 1],
                                lhsT=e_t[:, ch * 128 : (ch + 1) * 128],
                                rhs=ones81_sb,
                                start=True,
                                stop=True,
                            )
                        rd4 = spool.tile([128, 4], F32, tag="rd4")
                        ch0 = j0 // 128
                        if ch0 > 0:
                            nc.vector.memset(rd4[:, 0:ch0], 0.0)
                        nc.vector.reciprocal(
                            out=rd4[:, ch0:], in_=den4_ps[:, ch0:]
                        )

                    # ---- K / Q linears -> bf16. K's bias is dropped: it adds
                    # a per-query constant to scores, invariant under softmax.
                    k_ps = linear("k")
                    kt_bf = spool.tile([128, SL], BF16, tag="kt")
                    nc.vector.tensor_copy(out=kt_bf, in_=k_ps)
                    q_ps = linear("q")
                    qt_bf = spool.tile([128, SL], BF16, tag="qt")
                    nc.scalar.activation(
                        out=qt_bf, in_=q_ps, func=AF.Identity, bias=b_sb["q"]
                    )

                    # ---- V linear -> bf16 VT -> transpose to V natural ----
                    v_ps = linear("v")
                    vt_bf = spool.tile([128, SL], BF16, tag="vt")
                    nc.scalar.activation(
                        out=vt_bf, in_=v_ps, func=AF.Identity, bias=b_sb["v"]
                    )
                    # vext[tok, (pr, d|1)]: pair pr rows 0:64 = even t, 64:128 odd
                    vx_ps = psV.tile([128, 4, 132], BF16, tag="vx", name="vx_ps")
                    for pr in range(4):
                        nc.tensor.transpose(
                            out=vx_ps[:, pr, 0:128],
                            in_=vt_bf[:, pr * 128 : (pr + 1) * 128],
                            identity=idb_sb,
                        )
                    vext = spool.tile([128, 4, 132], BF16, tag="vnat")
                    nc.scalar.copy(out=vext[:, :, 0:128], in_=vx_ps[:, :, 0:128])
                    nc.vector.memset(vext[:, :, 128:129], 1.0)

                    # ---- attention: 4 pairs of timesteps, scores [k, q] ----
                    # att tile regions: [0:128]=sc1, [128:256]=sc2, [256:385]=o|den
                    out_sb = spool.tile([128, 4, DK], F32, tag="osb")
                    for pr in range(4):
                        c1 = pr * 128
                        att = psA.tile([128, 512], F32, tag="att", name="att")
                        nc.tensor.matmul(
                            att[:, 0:128],
                            lhsT=kt_bf[:, c1 : c1 + 128],
                            rhs=qt_bf[:, c1 : c1 + 128],
                            start=True,
                            stop=True,
                        )
                        attn_bf = attn_ring[(c * 4 + pr) % 3]
                        if rd4 is not None:
                            nc.tensor.matmul(
                                att[:, 128:256],
                                lhsT=nt_bf[:, c1 : c1 + 128],
                                rhs=qt_bf[:, c1 : c1 + 128],
                                start=True,
                                stop=True,
                            )
                            sc2s = spool.tile([128, 128], BF16, tag="sc2s")
                            nc.vector.tensor_scalar_mul(
                                out=sc2s,
                                in0=att[:, 128:256],
                                scalar1=rd4[:, pr : pr + 1],
                            )
                            sccmb = spool.tile([128, 128], BF16, tag="sccmb")
                            nc.vector.tensor_tensor(
                                out=sccmb,
                                in0=att[:, 0:128],
                                in1=sc2s,
                                op=ALU.add,
                            )
                            for h in range(2):
                                r0 = 64 * h
                                nc.scalar.activation(
                                    out=attn_bf[r0 : r0 + 64, r0 : r0 + 64],
                                    in_=sccmb[r0 : r0 + 64, r0 : r0 + 64],
                                    func=AF.Exp,
                                    scale=scale,
                                )
                        else:
                            for h in range(2):
                                r0 = 64 * h
                                nc.scalar.activation(
                                    out=attn_bf[r0 : r0 + 64, r0 : r0 + 64],
                                    in_=att[r0 : r0 + 64, r0 : r0 + 64],
                                    func=AF.Exp,
                                    scale=scale,
                                )
                        nc.tensor.matmul(
                            att[:, 256:385],
                            lhsT=attn_bf,
                            rhs=vext[:, pr, 0:129],
                            start=True,
                            stop=True,
                        )
                        rs = spool.tile([128, 1], F32, tag="rs")
                        nc.vector.reciprocal(out=rs, in_=att[:, 384:385])
                        nc.vector.tensor_scalar_mul(
                            out=out_sb[:, pr, :], in0=att[:, 256:384], scalar1=rs
                        )

                    nc.sync.dma_start(
                        out=out_flat[b, tok0 : tok0 + SL, :].rearrange(
                            "(j p) d -> p j d", p=128
                        ),
                        in_=out_sb,
                    )
    nc.finalize()
    return nc


def _host_prep(inputs: dict) -> dict:
    f = np.float32
    bf = ml_dtypes.bfloat16
    aux = {}
    for k, (W, bias) in {
        "q": (inputs["WQ"], inputs["bQ"]),
        "k": (inputs["WK"], inputs["bK"]),
        "v": (inputs["WV"], inputs["bV"]),
        "u": (inputs["Wu"], inputs["bu"]),
    }.items():
        aux[f"wt{k}"] = np.ascontiguousarray(
            np.asarray(W, f).T.reshape(2, 128, DK)
        ).astype(bf)
        aux[f"b{k}"] = np.ascontiguousarray(np.asarray(bias, f).reshape(DK, 1))
    patterns = np.asarray(inputs["patterns"], f)
    m = patterns @ np.asarray(inputs["Wm"], f).T + np.asarray(inputs["bm"], f)
    aux["mT"] = np.ascontiguousarray(
        m.transpose(2, 1, 0).reshape(DK, S_WIN * N_PAT)
    ).astype(bf)
    aux["csum"] = np.ascontiguousarray(
        (patterns @ np.asarray(inputs["Wc"], f).T + np.asarray(inputs["bc"], f)).sum(
            axis=1
        )
    ).astype(bf)
    aux["idb"] = np.eye(128, dtype=bf)
    aux["ones81"] = np.ones([N_PAT, 1], bf)
    return aux


TRACE = False
LAST_RESULTS = None


def kernel(**inputs) -> np.ndarray:
    global LAST_RESULTS
    from concourse.bass_utils import run_bass_kernel_spmd

    x = np.asarray(inputs["x"], np.float32)
    B, T = x.shape[0], x.shape[1]
    bs = B // N_CORES
    x_bf = x.astype(ml_dtypes.bfloat16)
    aux = _host_prep(inputs)
    nc = build_program(bs, T)
    in_maps = [dict(aux, x=x_bf[i * bs : (i + 1) * bs]) for i in range(N_CORES)]
    res = run_bass_kernel_spmd(nc, in_maps, list(range(N_CORES)), trace=TRACE)
    LAST_RESULTS = res
    return np.concatenate([r["out"] for r in res.results], axis=0)
